# revision 3
# baseline (speedup 1.0000x reference)
"""PixelCNN gated-stack forward pass on 8 Trainium2 NeuronCores.

Strategy (pure data parallelism): 16 images are sharded 2-per-core; all
weights are replicated. On each core the whole 16-layer gated PixelCNN runs
out of SBUF:

- Activations live in zero-padded SBUF planes (stride 66 = 64 cols + 2 gap
  cols, 2 pad rows on top of each 64-row image) so every conv tap is just a
  matmul against a shifted access pattern of the same buffer; the pad/gap
  zeros implement conv zero-padding for free.
- Every conv/1x1 is a K<=128 matmul accumulated in PSUM over taps, N=512
  (8 image rows) per matmul. Weights (fp16) are pre-transposed on the host
  into matmul-ready [K, M] layouts; channel-causality masks, the even/odd
  gating split, bias folding (v2h @ vconv_b) and the final (C,256)->(256,C)
  output permutation are all baked into the host-side weight prep.
- Layer 0 (7x7 causal conv over only 4 input channels) is done with a
  host-built im2col buffer: 112 partitions = (4 row-shifts x 7 col-shifts x
  4 channels), so the whole 7x7 conv is 2 matmuls per output half, and the
  width-6 horizontal conv + masked 1x1 fuse into a single K=28 matmul.
- Gating tanh(a)*sigmoid(b) runs on the scalar (ACT) engine with
  per-partition biases fused into the activation; multiplies, PSUM->SBUF
  fp16 casts and the residual accumulation run on the vector (DVE) engine.
- fp16 matmul operands / fp32 PSUM accumulate (measured end-to-end error
  vs the fp32 reference ~5e-4 relative).
"""

import os
import numpy as np

C, H, W, DIM, NL = 3, 64, 64, 192, 16
HALF = DIM // 2  # 96
N_IMG, N_CORES, IPC = 16, 8, 2
S, RV = W + 2, H + 2          # padded plane stride / rows for layer buffers
PL = RV * S                    # 4356 elements per image plane
LEAD = 2                       # leading guard zeros before first plane
TRAIL = S                      # trailing slack so 8-row rearrange spans stay in range
S0, R0 = W + 6, H + 7          # layer-0 im2col plane geometry (7x7 conv)
PL0 = R0 * S0                  # 4970
NPOS = 512                     # matmul moving-dim: 8 rows x 64 cols
NSITE = IPC * 8                # 16 sites (image, 8-row block) per core
NLAYERS = int(os.environ.get("KERNEL_NLAYERS", str(NL)))


def _subpix_mask(out_dim, in_dim, c, mask_type):
    i_sub, o_sub = in_dim // c, out_dim // c
    m = np.zeros((out_dim, in_dim), np.float32)
    for g in range(c):
        ncols = i_sub * (g + (1 if mask_type == 'B' else 0))
        if ncols:
            m[o_sub * g:o_sub * (g + 1), :ncols] = 1.0
    return m


def _asnp(v):
    return np.asarray(v, dtype=np.float32)


def prep_host(inputs):
    """Host-side weight/input preparation -> dict of DMA-ready numpy arrays."""
    inp = {k: ([_asnp(x) for x in v] if isinstance(v, (list, tuple)) else _asnp(v))
           for k, v in inputs.items()}
    f16 = np.float16
    in_dims = [C + 1] + [HALF] * (NL - 1)
    sub_masks = [_subpix_mask(DIM, in_dims[i], C, 'A' if i == 0 else 'B')
                 for i in range(NL)]
    mask_h = _subpix_mask(HALF, HALF, C, 'B')
    mask_o2 = _subpix_mask(C * 256, HALF, C, 'B')
    perm = np.concatenate([np.arange(0, DIM, 2), np.arange(1, DIM, 2)])  # e|o

    vw = np.zeros((NL - 1, HALF, 9 * DIM), f16)
    hw = np.zeros((NL - 1, HALF, 3 * DIM), f16)
    v2h = np.zeros((NL, HALF, 2 * DIM), f16)
    hact = np.zeros((NL, HALF, HALF), f16)
    bias96 = np.zeros((HALF, 81), np.float32)
    for i in range(NL):
        if i > 0:
            wp = inp['vconv_w'][i][perm]                      # (192,96,3,3)
            vw[i - 1] = wp.transpose(1, 2, 3, 0).reshape(HALF, 9 * DIM)
            hp = inp['hconv_w'][i][perm]                      # (192,96,1,2)
            hs = (inp['hsub_w'][i] * sub_masks[i])[perm]      # (192,96)
            hw[i - 1] = np.stack(
                [hp[:, :, 0, 0].T, hp[:, :, 0, 1].T, hs.T], axis=1
            ).reshape(HALF, 3 * DIM)
        vp = inp['v2h_w'][i][perm]                            # (192,192)
        v2h[i] = np.stack([vp[:, 0::2].T, vp[:, 1::2].T], axis=1) \
            .reshape(HALF, 2 * DIM)
        hact[i] = (inp['hact_w'][i] * mask_h).T
        bias96[:, 2 * i] = inp['vconv_b'][i][0::2]
        bias96[:, 2 * i + 1] = inp['vconv_b'][i][1::2]
        hbf = inp['hconv_b'][i] + inp['v2h_b'][i] + inp['v2h_w'][i] @ inp['vconv_b'][i]
        bias96[:, 32 + 2 * i] = hbf[0::2]
        bias96[:, 33 + 2 * i] = hbf[1::2]
        bias96[:, 64 + i] = inp['hact_b'][i]
    bias96[:, 80] = inp['out1_b']

    w0p = inp['vconv_w'][0][perm]                             # (192,4,7,7)
    a0 = w0p.transpose(2, 3, 1, 0)                            # [ky,kx,c,out]
    l0v1 = a0[0:4].reshape(112, DIM).astype(f16)
    l0v2 = a0[4:7].reshape(84, DIM).astype(f16)
    h0p = inp['hconv_w'][0][perm]                             # (192,4,1,6)
    hs0 = (inp['hsub_w'][0] * sub_masks[0])[perm]             # (192,4)
    l0h = np.concatenate(
        [h0p[:, :, 0, :].transpose(2, 1, 0).reshape(24, DIM), hs0.T.reshape(4, DIM)],
        axis=0).astype(f16)                                   # (28,192)

    o1w = ((inp['out1_w'] * mask_h).T).astype(f16)            # (96,96)
    perm2 = np.array([(m % 3) * 256 + m // 3 for m in range(C * 256)])
    o2w = ((inp['out2_w'] * mask_o2)[perm2].T).astype(f16)    # (96,768)
    o2b = inp['out2_b'][perm2].reshape(6, 128).T.copy()       # (128,6) fp32

    # layer-0 im2col: [img, 112, PL0]
    x = inp['x']
    xn = (x / 127.5 - 1.0) * 2.0
    src = np.zeros((N_IMG, 4, R0, S0), np.float32)
    src[:, :3, 7:, :W] = xn
    src[:, 3, 7:, :W] = 1.0
    srcp = np.pad(src.reshape(N_IMG, 4, PL0), ((0, 0), (0, 0), (3, 220)))
    im2col = np.zeros((N_IMG, 112, PL0), f16)
    for dyj in range(4):
        for dxi in range(7):
            d = dyj * S0 + dxi - 3
            q0 = dyj * 28 + dxi * 4
            im2col[:, q0:q0 + 4] = srcp[:, :, 3 + d:3 + d + PL0]

    out = {
        'vw': vw.reshape(NL - 1, HALF, 9 * DIM),
        'hw': hw, 'v2h': v2h, 'hact': hact, 'bias96': bias96,
        'l0v1': l0v1, 'l0v2': l0v2, 'l0h': l0h,
        'o1w': o1w, 'o2w': o2w, 'o2b': o2b,
    }
    x0_cores = []
    for cidx in range(N_CORES):
        sl = im2col[IPC * cidx: IPC * (cidx + 1)]             # [2,112,PL0]
        x0_cores.append(np.ascontiguousarray(
            sl.transpose(1, 0, 2).reshape(112, IPC * PL0)))
    return out, x0_cores


_BUILD_CACHE = {}


def build_nc(debug_dump=False):
    key = (NLAYERS, debug_dump)
    if key in _BUILD_CACHE:
        return _BUILD_CACHE[key]
    import concourse.bass as bass  # noqa: F401
    from concourse import bacc
    import concourse.mybir as mybir
    from concourse.tile import TileContext

    f16, f32 = mybir.dt.float16, mybir.dt.float32
    AF = mybir.ActivationFunctionType
    nc = bacc.Bacc(trn_type="TRN2")

    d_x0 = nc.dram_tensor("x0", [112, IPC * PL0], f16, kind="ExternalInput")
    d_vw = nc.dram_tensor("vw", [NL - 1, HALF, 9 * DIM], f16, kind="ExternalInput")
    d_hw = nc.dram_tensor("hw", [NL - 1, HALF, 3 * DIM], f16, kind="ExternalInput")
    d_v2h = nc.dram_tensor("v2h", [NL, HALF, 2 * DIM], f16, kind="ExternalInput")
    d_hact = nc.dram_tensor("hact", [NL, HALF, HALF], f16, kind="ExternalInput")
    d_b96 = nc.dram_tensor("bias96", [HALF, 81], f32, kind="ExternalInput")
    d_l0v1 = nc.dram_tensor("l0v1", [112, DIM], f16, kind="ExternalInput")
    d_l0v2 = nc.dram_tensor("l0v2", [84, DIM], f16, kind="ExternalInput")
    d_l0h = nc.dram_tensor("l0h", [28, DIM], f16, kind="ExternalInput")
    d_o1w = nc.dram_tensor("o1w", [HALF, HALF], f16, kind="ExternalInput")
    d_o2w = nc.dram_tensor("o2w", [HALF, C * 256], f16, kind="ExternalInput")
    d_o2b = nc.dram_tensor("o2b", [128, 6], f32, kind="ExternalInput")
    d_out = nc.dram_tensor("out", [IPC, C * 256, H * W], f32, kind="ExternalOutput")
    if debug_dump:
        d_dvx = nc.dram_tensor("dbg_vx", [HALF, LEAD + IPC * PL + TRAIL], f16,
                               kind="ExternalOutput")
        d_dhx = nc.dram_tensor("dbg_hx", [HALF, LEAD + IPC * PL + TRAIL], f16,
                               kind="ExternalOutput")

    with TileContext(nc) as tc:
        with tc.tile_pool(name="const", bufs=1) as cpool, \
             tc.tile_pool(name="wts", bufs=2) as wpool, \
             tc.tile_pool(name="scr", bufs=2) as spool, \
             tc.tile_pool(name="psum", bufs=2, space="PSUM") as ppool:

            vx16 = cpool.tile([HALF, LEAD + IPC * PL + TRAIL], f16, name="vx16", tag="vx16")
            hx16 = cpool.tile([HALF, LEAD + IPC * PL + TRAIL], f16, name="hx16", tag="hx16")
            x0 = cpool.tile([112, IPC * PL0], f16, name="x0t", tag="x0t")
            l0v1 = cpool.tile([112, DIM], f16, name="l0v1t", tag="l0v1t")
            l0v2 = cpool.tile([84, DIM], f16, name="l0v2t", tag="l0v2t")
            l0h = cpool.tile([28, DIM], f16, name="l0ht", tag="l0ht")
            o1w = cpool.tile([HALF, HALF], f16, name="o1wt", tag="o1wt")
            o2w = cpool.tile([HALF, C * 256], f16, name="o2wt", tag="o2wt")
            b96 = cpool.tile([HALF, 81], f32, name="b96t", tag="b96t")
            o2b = cpool.tile([128, 6], f32, name="o2bt", tag="o2bt")

            nc.vector.memset(vx16[:, :], 0.0)
            nc.vector.memset(hx16[:, :], 0.0)
            nc.sync.dma_start(x0[:, :], d_x0[:, :])
            nc.sync.dma_start(l0v1[:, :], d_l0v1[:, :])
            nc.sync.dma_start(l0v2[:, :], d_l0v2[:, :])
            nc.sync.dma_start(l0h[:, :], d_l0h[:, :])
            nc.sync.dma_start(o1w[:, :], d_o1w[:, :])
            nc.sync.dma_start(o2w[:, :], d_o2w[:, :])
            nc.sync.dma_start(b96[:, :], d_b96[:, :])
            nc.sync.dma_start(o2b[:, :], d_o2b[:, :])

            warm16 = cpool.tile([HALF, NPOS], f16, name="warm16", tag="warm16")
            nc.vector.memset(warm16[:, :], 0.0)
            for wi in range(30):
                pwarm = ppool.tile([HALF, NPOS], f32, name=f"pwarm{wi}", tag="pve")
                nc.tensor.matmul(pwarm[:, :], warm16[:, 0:HALF], warm16[:, :],
                                 start=True, stop=True)

            def site_ap(buf, nparts, base, stride, ncol=W, nrow=8):
                sl = buf[0:nparts, base:base + nrow * stride]
                return sl.rearrange("p (r s) -> p r s", r=nrow, s=stride)[:, :, 0:ncol]

            def layer(i, vwt, hwt, v2ht, hactt, last=False):
                for s in reversed(range(NSITE)):
                    img, a = divmod(s, 8)
                    vb = LEAD + img * PL + (2 + 8 * a) * S
                    xb = img * PL0 + 8 * a * S0
                    pve = ppool.tile([HALF, NPOS], f32, name=f"pve{i}_{s}", tag="pve")
                    pvo = ppool.tile([HALF, NPOS], f32, name=f"pvo{i}_{s}", tag="pvo")
                    if i == 0:
                        for half, pp in ((0, pve), (1, pvo)):
                            m0 = half * HALF
                            nc.tensor.matmul(
                                pp[:, :], l0v1[:, m0:m0 + HALF],
                                site_ap(x0, 112, xb, S0),
                                start=True, stop=False)
                            nc.tensor.matmul(
                                pp[:, :], l0v2[:, m0:m0 + HALF],
                                site_ap(x0, 84, xb + 4 * S0, S0),
                                start=False, stop=True)
                    else:
                        t9 = 0
                        for dy in range(3):
                            for dx in range(3):
                                off = (dy - 2) * S + (dx - 1)
                                rhs = site_ap(vx16, HALF, vb + off, S)
                                nc.tensor.matmul(
                                    pve[:, :], vwt[:, t9 * DIM:t9 * DIM + HALF],
                                    rhs, start=(t9 == 0), stop=(t9 == 8))
                                t9 += 1
                        t9 = 0
                        for dy in range(3):
                            for dx in range(3):
                                off = (dy - 2) * S + (dx - 1)
                                rhs = site_ap(vx16, HALF, vb + off, S)
                                nc.tensor.matmul(
                                    pvo[:, :],
                                    vwt[:, t9 * DIM + HALF:(t9 + 1) * DIM],
                                    rhs, start=(t9 == 0), stop=(t9 == 8))
                                t9 += 1
                    ve = spool.tile([HALF, NPOS], f16, name=f"ve{i}_{s}", tag="ve")
                    vo = spool.tile([HALF, NPOS], f16, name=f"vo{i}_{s}", tag="vo")
                    nc.vector.tensor_copy(ve[:, :], pve[:, :])
                    nc.vector.tensor_copy(vo[:, :], pvo[:, :])
                    tt = spool.tile([HALF, NPOS], f32, name=f"tt{i}_{s}", tag="tt")
                    ss = spool.tile([HALF, NPOS], f32, name=f"ss{i}_{s}", tag="ss")
                    nc.scalar.activation(tt[:, :], pve[:, :], AF.Tanh,
                                         bias=b96[:, 2 * i:2 * i + 1])
                    nc.scalar.activation(ss[:, :], pvo[:, :], AF.Sigmoid,
                                         bias=b96[:, 2 * i + 1:2 * i + 2])
                    nc.vector.tensor_mul(site_ap(vx16, HALF, vb, S),
                                         tt[:, :], ss[:, :])

                    phe = ppool.tile([HALF, NPOS], f32, name=f"phe{i}_{s}", tag="phe")
                    pho = ppool.tile([HALF, NPOS], f32, name=f"pho{i}_{s}", tag="pho")
                    for half, pp in ((0, phe), (1, pho)):
                        m0 = half * HALF
                        if i == 0:
                            nc.tensor.matmul(
                                pp[:, :], l0h[:, m0:m0 + HALF],
                                site_ap(x0, 28, xb + 7 * S0 - 3, S0),
                                start=True, stop=False)
                        else:
                            for t3, off in enumerate((-2, -1, 0)):
                                rhs = site_ap(hx16, HALF, vb + off, S)
                                nc.tensor.matmul(
                                    pp[:, :], hwt[:, t3 * DIM + m0:t3 * DIM + m0 + HALF],
                                    rhs, start=(t3 == 0), stop=False)
                        nc.tensor.matmul(pp[:, :], v2ht[:, m0:m0 + HALF],
                                         ve[:, :], start=False, stop=False)
                        nc.tensor.matmul(pp[:, :], v2ht[:, DIM + m0:DIM + m0 + HALF],
                                         vo[:, :], start=False, stop=True)
                    gt = spool.tile([HALF, NPOS], f32, name=f"gt{i}_{s}", tag="gt")
                    gs = spool.tile([HALF, NPOS], f32, name=f"gs{i}_{s}", tag="gs")
                    nc.scalar.activation(gt[:, :], phe[:, :], AF.Tanh,
                                         bias=b96[:, 32 + 2 * i:33 + 2 * i])
                    nc.scalar.activation(gs[:, :], pho[:, :], AF.Sigmoid,
                                         bias=b96[:, 33 + 2 * i:34 + 2 * i])
                    g16 = spool.tile([HALF, NPOS], f16, name=f"g16{i}_{s}", tag="g16")
                    nc.vector.tensor_mul(g16[:, :], gt[:, :], gs[:, :])
                    phx = ppool.tile([HALF, NPOS], f32, name=f"phx{i}_{s}", tag="pve")
                    nc.tensor.matmul(phx[:, :], hactt[:, :], g16[:, :],
                                     start=True, stop=True)
                    hx_site = site_ap(hx16, HALF, vb, S)
                    if i == 0:
                        nc.vector.tensor_scalar_add(hx_site, phx[:, :],
                                                    b96[:, 64 + i:65 + i])
                    else:
                        nc.vector.scalar_tensor_tensor(
                            hx_site, phx[:, :], b96[:, 64 + i:65 + i], hx_site,
                            op0=mybir.AluOpType.add, op1=mybir.AluOpType.add)
                    if last:
                        head_site(s)

            def head_site(s):
                img, a = divmod(s, 8)
                vb = LEAD + img * PL + (2 + 8 * a) * S
                psum_tags = ["pvo", "phe", "pho", "pve", "pvo", "phe"]
                po1 = ppool.tile([HALF, NPOS], f32, name=f"po1_{s}", tag="pve")
                nc.tensor.matmul(po1[:, :], o1w[:, :],
                                 site_ap(hx16, HALF, vb, S), start=True, stop=True)
                osb = spool.tile([HALF, NPOS], f16, name=f"osb{s}", tag="osb")
                nc.scalar.activation(osb[:, :], po1[:, :], AF.Relu,
                                     bias=b96[:, 80:81])
                for t in range(6):
                    pl2 = ppool.tile([128, NPOS], f32, name=f"pl{s}_{t}",
                                     tag=psum_tags[t])
                    nc.tensor.matmul(pl2[:, :], o2w[:, 128 * t:128 * (t + 1)],
                                     osb[:, :], start=True, stop=True)
                    stg = spool.tile([128, NPOS], f32, name=f"stg{s}_{t}",
                                     tag="stg", bufs=4)
                    if t % 2 == 0:
                        nc.vector.tensor_scalar_add(stg[:, :], pl2[:, :],
                                                    o2b[:, t:t + 1])
                    else:
                        nc.scalar.add(stg[:, :], pl2[:, :], o2b[:, t:t + 1])
                    nc.sync.dma_start(
                        d_out[img, 128 * t:128 * (t + 1), 512 * a:512 * (a + 1)],
                        stg[:, :])

            for i in range(NLAYERS):
                if i > 0:
                    vwt = wpool.tile([HALF, 9 * DIM], f16, name=f"vw{i}", tag="vw")
                    hwt = wpool.tile([HALF, 3 * DIM], f16, name=f"hw{i}", tag="hw")
                    nc.sync.dma_start(vwt[:, :], d_vw[i - 1, :, :])
                    nc.sync.dma_start(hwt[:, :], d_hw[i - 1, :, :])
                else:
                    vwt = hwt = None
                v2ht = wpool.tile([HALF, 2 * DIM], f16, name=f"v2h{i}", tag="v2h")
                hactt = wpool.tile([HALF, HALF], f16, name=f"hact{i}", tag="hact")
                nc.sync.dma_start(v2ht[:, :], d_v2h[i, :, :])
                nc.sync.dma_start(hactt[:, :], d_hact[i, :, :])
                layer(i, vwt, hwt, v2ht, hactt, last=(i == NLAYERS - 1))
            if debug_dump:
                nc.sync.dma_start(d_dvx[:, :], vx16[:, :])
                nc.sync.dma_start(d_dhx[:, :], hx16[:, :])

    nc.compile()
    _BUILD_CACHE[key] = nc
    return nc


def kernel(**inputs):
    from concourse.bass_utils import run_bass_kernel_spmd
    shared, x0_cores = prep_host(inputs)
    debug = bool(int(os.environ.get("KERNEL_DEBUG_DUMP", "0")))
    nc = build_nc(debug_dump=debug)
    in_maps = [{**shared, 'x0': x0_cores[cidx]} for cidx in range(N_CORES)]
    res = run_bass_kernel_spmd(nc, in_maps, core_ids=list(range(N_CORES)))
    outs = np.concatenate([res.results[cidx]['out'] for cidx in range(N_CORES)],
                          axis=0)                              # [16,768,4096]
    if debug:
        kernel._dbg = [(res.results[cidx].get('dbg_vx'),
                        res.results[cidx].get('dbg_hx')) for cidx in range(N_CORES)]
        kernel._res = res
    return outs.reshape(N_IMG, 256, C, H, W)


# revision 4
# speedup vs baseline: 1.0263x; 1.0263x over previous
"""PixelCNN gated-stack forward pass on 8 Trainium2 NeuronCores.

Strategy (pure data parallelism): 16 images are sharded 2-per-core; all
weights are replicated. On each core the whole 16-layer gated PixelCNN runs
out of SBUF:

- Activations live in zero-padded SBUF planes (stride 66 = 64 cols + 2 gap
  cols, 2 pad rows on top of each 64-row image) so every conv tap is just a
  matmul against a shifted access pattern of the same buffer; the pad/gap
  zeros implement conv zero-padding for free.
- Every conv/1x1 is a K<=128 matmul accumulated in PSUM over taps, N=512
  (8 image rows) per matmul. Weights (fp16) are pre-transposed on the host
  into matmul-ready [K, M] layouts; channel-causality masks, the even/odd
  gating split, bias folding (v2h @ vconv_b) and the final (C,256)->(256,C)
  output permutation are all baked into the host-side weight prep.
- Layer 0 (7x7 causal conv over only 4 input channels) is done with a
  host-built im2col buffer: 112 partitions = (4 row-shifts x 7 col-shifts x
  4 channels), so the whole 7x7 conv is 2 matmuls per output half, and the
  width-6 horizontal conv + masked 1x1 fuse into a single K=28 matmul.
- Gating tanh(a)*sigmoid(b) runs on the scalar (ACT) engine with
  per-partition biases fused into the activation; multiplies, PSUM->SBUF
  fp16 casts and the residual accumulation run on the vector (DVE) engine.
- fp16 matmul operands / fp32 PSUM accumulate (measured end-to-end error
  vs the fp32 reference ~5e-4 relative).
"""

import os
import numpy as np

C, H, W, DIM, NL = 3, 64, 64, 192, 16
HALF = DIM // 2  # 96
N_IMG, N_CORES, IPC = 16, 8, 2
S, RV = W + 2, H + 2          # padded plane stride / rows for layer buffers
PL = RV * S                    # 4356 elements per image plane
LEAD = 2                       # leading guard zeros before first plane
TRAIL = S                      # trailing slack so 8-row rearrange spans stay in range
S0, R0 = W + 6, H + 7          # layer-0 im2col plane geometry (7x7 conv)
PL0 = R0 * S0                  # 4970
NPOS = 512                     # matmul moving-dim: 8 rows x 64 cols
NSITE = IPC * 8                # 16 sites (image, 8-row block) per core
NLAYERS = int(os.environ.get("KERNEL_NLAYERS", str(NL)))


def _subpix_mask(out_dim, in_dim, c, mask_type):
    i_sub, o_sub = in_dim // c, out_dim // c
    m = np.zeros((out_dim, in_dim), np.float32)
    for g in range(c):
        ncols = i_sub * (g + (1 if mask_type == 'B' else 0))
        if ncols:
            m[o_sub * g:o_sub * (g + 1), :ncols] = 1.0
    return m


def _asnp(v):
    return np.asarray(v, dtype=np.float32)


def prep_host(inputs):
    """Host-side weight/input preparation -> dict of DMA-ready numpy arrays."""
    inp = {k: ([_asnp(x) for x in v] if isinstance(v, (list, tuple)) else _asnp(v))
           for k, v in inputs.items()}
    f16 = np.float16
    in_dims = [C + 1] + [HALF] * (NL - 1)
    sub_masks = [_subpix_mask(DIM, in_dims[i], C, 'A' if i == 0 else 'B')
                 for i in range(NL)]
    mask_h = _subpix_mask(HALF, HALF, C, 'B')
    mask_o2 = _subpix_mask(C * 256, HALF, C, 'B')
    perm = np.concatenate([np.arange(0, DIM, 2), np.arange(1, DIM, 2)])  # e|o

    vw = np.zeros((NL - 1, HALF, 9 * DIM), f16)
    hw = np.zeros((NL - 1, HALF, 3 * DIM), f16)
    v2h = np.zeros((NL, HALF, 2 * DIM), f16)
    hact = np.zeros((NL, HALF, HALF), f16)
    bias96 = np.zeros((HALF, 81), np.float32)
    for i in range(NL):
        if i > 0:
            wp = inp['vconv_w'][i][perm]                      # (192,96,3,3)
            vw[i - 1] = wp.transpose(1, 2, 3, 0).reshape(HALF, 9 * DIM)
            hp = inp['hconv_w'][i][perm]                      # (192,96,1,2)
            hs = (inp['hsub_w'][i] * sub_masks[i])[perm]      # (192,96)
            hw[i - 1] = np.stack(
                [hp[:, :, 0, 0].T, hp[:, :, 0, 1].T, hs.T], axis=1
            ).reshape(HALF, 3 * DIM)
        vp = inp['v2h_w'][i][perm]                            # (192,192)
        v2h[i] = np.stack([vp[:, 0::2].T, vp[:, 1::2].T], axis=1) \
            .reshape(HALF, 2 * DIM)
        hact[i] = (inp['hact_w'][i] * mask_h).T
        bias96[:, 2 * i] = inp['vconv_b'][i][0::2]
        bias96[:, 2 * i + 1] = inp['vconv_b'][i][1::2]
        hbf = inp['hconv_b'][i] + inp['v2h_b'][i] + inp['v2h_w'][i] @ inp['vconv_b'][i]
        bias96[:, 32 + 2 * i] = hbf[0::2]
        bias96[:, 33 + 2 * i] = hbf[1::2]
        bias96[:, 64 + i] = inp['hact_b'][i]
    bias96[:, 80] = inp['out1_b']

    w0p = inp['vconv_w'][0][perm]                             # (192,4,7,7)
    a0 = w0p.transpose(2, 3, 1, 0)                            # [ky,kx,c,out]
    l0v1 = a0[0:4].reshape(112, DIM).astype(f16)
    l0v2 = a0[4:7].reshape(84, DIM).astype(f16)
    h0p = inp['hconv_w'][0][perm]                             # (192,4,1,6)
    hs0 = (inp['hsub_w'][0] * sub_masks[0])[perm]             # (192,4)
    l0h = np.concatenate(
        [h0p[:, :, 0, :].transpose(2, 1, 0).reshape(24, DIM), hs0.T.reshape(4, DIM)],
        axis=0).astype(f16)                                   # (28,192)

    o1w = ((inp['out1_w'] * mask_h).T).astype(f16)            # (96,96)
    perm2 = np.array([(m % 3) * 256 + m // 3 for m in range(C * 256)])
    o2w = ((inp['out2_w'] * mask_o2)[perm2].T).astype(f16)    # (96,768)
    o2b = inp['out2_b'][perm2].reshape(6, 128).T.copy()       # (128,6) fp32

    # layer-0 im2col: [img, 112, PL0]
    x = inp['x']
    xn = (x / 127.5 - 1.0) * 2.0
    src = np.zeros((N_IMG, 4, R0, S0), np.float32)
    src[:, :3, 7:, :W] = xn
    src[:, 3, 7:, :W] = 1.0
    srcp = np.pad(src.reshape(N_IMG, 4, PL0), ((0, 0), (0, 0), (3, 220)))
    im2col = np.zeros((N_IMG, 112, PL0), f16)
    for dyj in range(4):
        for dxi in range(7):
            d = dyj * S0 + dxi - 3
            q0 = dyj * 28 + dxi * 4
            im2col[:, q0:q0 + 4] = srcp[:, :, 3 + d:3 + d + PL0]

    out = {
        'vw': vw.reshape(NL - 1, HALF, 9 * DIM),
        'hw': hw, 'v2h': v2h, 'hact': hact, 'bias96': bias96,
        'l0v1': l0v1, 'l0v2': l0v2, 'l0h': l0h,
        'o1w': o1w, 'o2w': o2w, 'o2b': o2b,
    }
    x0_cores = []
    for cidx in range(N_CORES):
        sl = im2col[IPC * cidx: IPC * (cidx + 1)]             # [2,112,PL0]
        x0_cores.append(np.ascontiguousarray(
            sl.transpose(1, 0, 2).reshape(112, IPC * PL0)))
    return out, x0_cores


_BUILD_CACHE = {}


def build_nc(debug_dump=False):
    key = (NLAYERS, debug_dump)
    if key in _BUILD_CACHE:
        return _BUILD_CACHE[key]
    import concourse.bass as bass  # noqa: F401
    from concourse import bacc
    import concourse.mybir as mybir
    from concourse.tile import TileContext

    f16, f32 = mybir.dt.float16, mybir.dt.float32
    AF = mybir.ActivationFunctionType
    nc = bacc.Bacc(trn_type="TRN2")

    d_x0 = nc.dram_tensor("x0", [112, IPC * PL0], f16, kind="ExternalInput")
    d_vw = nc.dram_tensor("vw", [NL - 1, HALF, 9 * DIM], f16, kind="ExternalInput")
    d_hw = nc.dram_tensor("hw", [NL - 1, HALF, 3 * DIM], f16, kind="ExternalInput")
    d_v2h = nc.dram_tensor("v2h", [NL, HALF, 2 * DIM], f16, kind="ExternalInput")
    d_hact = nc.dram_tensor("hact", [NL, HALF, HALF], f16, kind="ExternalInput")
    d_b96 = nc.dram_tensor("bias96", [HALF, 81], f32, kind="ExternalInput")
    d_l0v1 = nc.dram_tensor("l0v1", [112, DIM], f16, kind="ExternalInput")
    d_l0v2 = nc.dram_tensor("l0v2", [84, DIM], f16, kind="ExternalInput")
    d_l0h = nc.dram_tensor("l0h", [28, DIM], f16, kind="ExternalInput")
    d_o1w = nc.dram_tensor("o1w", [HALF, HALF], f16, kind="ExternalInput")
    d_o2w = nc.dram_tensor("o2w", [HALF, C * 256], f16, kind="ExternalInput")
    d_o2b = nc.dram_tensor("o2b", [128, 6], f32, kind="ExternalInput")
    d_out = nc.dram_tensor("out", [IPC, C * 256, H * W], f16, kind="ExternalOutput")
    if debug_dump:
        d_dvx = nc.dram_tensor("dbg_vx", [HALF, LEAD + IPC * PL + TRAIL], f16,
                               kind="ExternalOutput")
        d_dhx = nc.dram_tensor("dbg_hx", [HALF, LEAD + IPC * PL + TRAIL], f16,
                               kind="ExternalOutput")

    with TileContext(nc) as tc:
        with tc.tile_pool(name="const", bufs=1) as cpool, \
             tc.tile_pool(name="wts", bufs=2) as wpool, \
             tc.tile_pool(name="scr", bufs=2) as spool, \
             tc.tile_pool(name="psum", bufs=2, space="PSUM") as ppool:

            vx16 = cpool.tile([HALF, LEAD + IPC * PL + TRAIL], f16, name="vx16", tag="vx16")
            hx16 = cpool.tile([HALF, LEAD + IPC * PL + TRAIL], f16, name="hx16", tag="hx16")
            x0 = cpool.tile([112, IPC * PL0], f16, name="x0t", tag="x0t")
            l0v1 = cpool.tile([112, DIM], f16, name="l0v1t", tag="l0v1t")
            l0v2 = cpool.tile([84, DIM], f16, name="l0v2t", tag="l0v2t")
            l0h = cpool.tile([28, DIM], f16, name="l0ht", tag="l0ht")
            o1w = cpool.tile([HALF, HALF], f16, name="o1wt", tag="o1wt")
            o2w = cpool.tile([HALF, C * 256], f16, name="o2wt", tag="o2wt")
            b96 = cpool.tile([HALF, 81], f32, name="b96t", tag="b96t")
            o2b = cpool.tile([128, 6], f32, name="o2bt", tag="o2bt")

            nc.vector.memset(vx16[:, :], 0.0)
            nc.vector.memset(hx16[:, :], 0.0)
            nc.sync.dma_start(x0[:, PL0:2 * PL0], d_x0[:, PL0:2 * PL0])
            nc.sync.dma_start(x0[:, 0:PL0], d_x0[:, 0:PL0])
            nc.sync.dma_start(l0v1[:, :], d_l0v1[:, :])
            nc.sync.dma_start(l0v2[:, :], d_l0v2[:, :])
            nc.sync.dma_start(l0h[:, :], d_l0h[:, :])
            nc.sync.dma_start(o1w[:, :], d_o1w[:, :])
            nc.sync.dma_start(o2w[:, :], d_o2w[:, :])
            nc.sync.dma_start(b96[:, :], d_b96[:, :])
            nc.sync.dma_start(o2b[:, :], d_o2b[:, :])

            def site_ap(buf, nparts, base, stride, ncol=W, nrow=8):
                sl = buf[0:nparts, base:base + nrow * stride]
                return sl.rearrange("p (r s) -> p r s", r=nrow, s=stride)[:, :, 0:ncol]

            def layer(i, vwt, hwt, v2ht, hactt):
                for s in reversed(range(NSITE)):
                    img, a = divmod(s, 8)
                    vb = LEAD + img * PL + (2 + 8 * a) * S
                    xb = img * PL0 + 8 * a * S0
                    pve = ppool.tile([HALF, NPOS], f32, name=f"pve{i}_{s}", tag="pve")
                    pvo = ppool.tile([HALF, NPOS], f32, name=f"pvo{i}_{s}", tag="pvo")
                    if i == 0:
                        for half, pp in ((0, pve), (1, pvo)):
                            m0 = half * HALF
                            nc.tensor.matmul(
                                pp[:, :], l0v1[:, m0:m0 + HALF],
                                site_ap(x0, 112, xb, S0),
                                start=True, stop=False)
                            nc.tensor.matmul(
                                pp[:, :], l0v2[:, m0:m0 + HALF],
                                site_ap(x0, 84, xb + 4 * S0, S0),
                                start=False, stop=True)
                    else:
                        t9 = 0
                        for dy in range(3):
                            for dx in range(3):
                                off = (dy - 2) * S + (dx - 1)
                                rhs = site_ap(vx16, HALF, vb + off, S)
                                nc.tensor.matmul(
                                    pve[:, :], vwt[:, t9 * DIM:t9 * DIM + HALF],
                                    rhs, start=(t9 == 0), stop=(t9 == 8))
                                t9 += 1
                        t9 = 0
                        for dy in range(3):
                            for dx in range(3):
                                off = (dy - 2) * S + (dx - 1)
                                rhs = site_ap(vx16, HALF, vb + off, S)
                                nc.tensor.matmul(
                                    pvo[:, :],
                                    vwt[:, t9 * DIM + HALF:(t9 + 1) * DIM],
                                    rhs, start=(t9 == 0), stop=(t9 == 8))
                                t9 += 1
                    ve = spool.tile([HALF, NPOS], f16, name=f"ve{i}_{s}", tag="ve")
                    vo = spool.tile([HALF, NPOS], f16, name=f"vo{i}_{s}", tag="vo")
                    nc.vector.tensor_copy(ve[:, :], pve[:, :])
                    nc.vector.tensor_copy(vo[:, :], pvo[:, :])
                    tt = spool.tile([HALF, NPOS], f32, name=f"tt{i}_{s}", tag="tt")
                    ss = spool.tile([HALF, NPOS], f32, name=f"ss{i}_{s}", tag="ss")
                    nc.scalar.activation(tt[:, :], pve[:, :], AF.Tanh,
                                         bias=b96[:, 2 * i:2 * i + 1])
                    nc.scalar.activation(ss[:, :], pvo[:, :], AF.Sigmoid,
                                         bias=b96[:, 2 * i + 1:2 * i + 2])
                    nc.vector.tensor_mul(site_ap(vx16, HALF, vb, S),
                                         tt[:, :], ss[:, :])

                    phe = ppool.tile([HALF, NPOS], f32, name=f"phe{i}_{s}", tag="phe")
                    pho = ppool.tile([HALF, NPOS], f32, name=f"pho{i}_{s}", tag="pho")
                    for half, pp in ((0, phe), (1, pho)):
                        m0 = half * HALF
                        if i == 0:
                            nc.tensor.matmul(
                                pp[:, :], l0h[:, m0:m0 + HALF],
                                site_ap(x0, 28, xb + 7 * S0 - 3, S0),
                                start=True, stop=False)
                        else:
                            for t3, off in enumerate((-2, -1, 0)):
                                rhs = site_ap(hx16, HALF, vb + off, S)
                                nc.tensor.matmul(
                                    pp[:, :], hwt[:, t3 * DIM + m0:t3 * DIM + m0 + HALF],
                                    rhs, start=(t3 == 0), stop=False)
                        nc.tensor.matmul(pp[:, :], v2ht[:, m0:m0 + HALF],
                                         ve[:, :], start=False, stop=False)
                        nc.tensor.matmul(pp[:, :], v2ht[:, DIM + m0:DIM + m0 + HALF],
                                         vo[:, :], start=False, stop=True)
                    gt = spool.tile([HALF, NPOS], f32, name=f"gt{i}_{s}", tag="gt")
                    gs = spool.tile([HALF, NPOS], f32, name=f"gs{i}_{s}", tag="gs")
                    nc.scalar.activation(gt[:, :], phe[:, :], AF.Tanh,
                                         bias=b96[:, 32 + 2 * i:33 + 2 * i])
                    nc.scalar.activation(gs[:, :], pho[:, :], AF.Sigmoid,
                                         bias=b96[:, 33 + 2 * i:34 + 2 * i])
                    g16 = spool.tile([HALF, NPOS], f16, name=f"g16{i}_{s}", tag="g16")
                    nc.vector.tensor_mul(g16[:, :], gt[:, :], gs[:, :])
                    phx = ppool.tile([HALF, NPOS], f32, name=f"phx{i}_{s}", tag="pve")
                    nc.tensor.matmul(phx[:, :], hactt[:, :], g16[:, :],
                                     start=True, stop=True)
                    hx_site = site_ap(hx16, HALF, vb, S)
                    if i == 0:
                        nc.vector.tensor_scalar_add(hx_site, phx[:, :],
                                                    b96[:, 64 + i:65 + i])
                    else:
                        nc.vector.scalar_tensor_tensor(
                            hx_site, phx[:, :], b96[:, 64 + i:65 + i], hx_site,
                            op0=mybir.AluOpType.add, op1=mybir.AluOpType.add)

            def head_site(s):
                img, a = divmod(s, 8)
                vb = LEAD + img * PL + (2 + 8 * a) * S
                psum_tags = ["pvo", "phe", "pho", "pve", "pvo", "phe"]
                po1 = ppool.tile([HALF, NPOS], f32, name=f"po1_{s}", tag="pve")
                nc.tensor.matmul(po1[:, :], o1w[:, :],
                                 site_ap(hx16, HALF, vb, S), start=True, stop=True)
                osb = spool.tile([HALF, NPOS], f16, name=f"osb{s}", tag="osb")
                nc.scalar.activation(osb[:, :], po1[:, :], AF.Relu,
                                     bias=b96[:, 80:81])
                for t in range(6):
                    pl2 = ppool.tile([128, NPOS], f32, name=f"pl{s}_{t}",
                                     tag=psum_tags[t])
                    nc.tensor.matmul(pl2[:, :], o2w[:, 128 * t:128 * (t + 1)],
                                     osb[:, :], start=True, stop=True)
                    stg = spool.tile([128, NPOS], f16, name=f"stg{s}_{t}",
                                     tag="stg", bufs=4)
                    if t % 2 == 0:
                        nc.vector.tensor_scalar_add(stg[:, :], pl2[:, :],
                                                    o2b[:, t:t + 1])
                    else:
                        nc.scalar.add(stg[:, :], pl2[:, :], o2b[:, t:t + 1])
                    nc.sync.dma_start(
                        d_out[img, 128 * t:128 * (t + 1), 512 * a:512 * (a + 1)],
                        stg[:, :])

            for i in range(NLAYERS):
                if i > 0:
                    vwt = wpool.tile([HALF, 9 * DIM], f16, name=f"vw{i}", tag="vw")
                    hwt = wpool.tile([HALF, 3 * DIM], f16, name=f"hw{i}", tag="hw")
                    nc.sync.dma_start(vwt[:, :], d_vw[i - 1, :, :])
                    nc.sync.dma_start(hwt[:, :], d_hw[i - 1, :, :])
                else:
                    vwt = hwt = None
                v2ht = wpool.tile([HALF, 2 * DIM], f16, name=f"v2h{i}", tag="v2h")
                hactt = wpool.tile([HALF, HALF], f16, name=f"hact{i}", tag="hact")
                nc.sync.dma_start(v2ht[:, :], d_v2h[i, :, :])
                nc.sync.dma_start(hactt[:, :], d_hact[i, :, :])
                layer(i, vwt, hwt, v2ht, hactt)

            for s in reversed(range(NSITE)):
                head_site(s)
            if debug_dump:
                nc.sync.dma_start(d_dvx[:, :], vx16[:, :])
                nc.sync.dma_start(d_dhx[:, :], hx16[:, :])

    nc.compile()
    _BUILD_CACHE[key] = nc
    return nc


def kernel(**inputs):
    from concourse.bass_utils import run_bass_kernel_spmd
    shared, x0_cores = prep_host(inputs)
    debug = bool(int(os.environ.get("KERNEL_DEBUG_DUMP", "0")))
    nc = build_nc(debug_dump=debug)
    in_maps = [{**shared, 'x0': x0_cores[cidx]} for cidx in range(N_CORES)]
    res = run_bass_kernel_spmd(nc, in_maps, core_ids=list(range(N_CORES)))
    outs = np.concatenate([res.results[cidx]['out'] for cidx in range(N_CORES)],
                          axis=0).astype(np.float32)           # [16,768,4096]
    if debug:
        kernel._dbg = [(res.results[cidx].get('dbg_vx'),
                        res.results[cidx].get('dbg_hx')) for cidx in range(N_CORES)]
        kernel._res = res
    return outs.reshape(N_IMG, 256, C, H, W)
